# revision 1
# baseline (speedup 1.0000x reference)
"""Bahdanau-attention scoring kernel for 8 TRN2 NeuronCores.

Reference computation (S=2048, B=32, H=1024):
    cat    = concat([broadcast(hidden), enc], axis=2)          # [S,B,2H]
    alphas = tanh(einsum('sbk,hk->sbh', cat, W_attn) + b_attn) # [S,B,H]
    scores = einsum('sbh,h->sb', alphas, v)                    # [S,B]
    out    = softmax(scores.T, axis=1)[:, None, :]             # [B,1,S]

Because hidden is broadcast over S, the concat-matmul splits into
    z[s,b,:] = W2 @ enc[s,b,:] + (W1 @ hidden[b,:] + b_attn)
with W1 = W_attn[:, :H], W2 = W_attn[:, H:].  The per-batch term hp[b,:]
is a [H]-vector, computed once on device and broadcast to all partitions.

Layout: s on partitions, h on the free dim.  Per 128-row s-tile:
  z = et.T @ W2T-slice (8 accumulating matmuls, enc tile stationary),
  zq = z + hp[b] (DVE), alphas = tanh(zq) (ACT),
  scores partial = sum_h alphas*v (DVE tensor_tensor_reduce).
This keeps the TensorEngine to exactly the 137 GFLOP main matmul; the
v-contraction and bias ride on the otherwise-idle Vector engine.

Sharding: data-parallel over batch. Core c handles batches 4c..4c+3.
"""

import sys

for _p in ("/opt/trn_rl_repo", "/root/.axon_site/_ro/trn_rl_repo"):
    if _p not in sys.path:
        sys.path.insert(0, _p)

import numpy as np
import ml_dtypes

import concourse.bass as bass  # noqa: F401  (bass must import before tile)
import concourse.mybir as mybir
import concourse.tile as tile
from concourse import bacc
from concourse.bass_isa import ReduceOp
from concourse.bass_utils import run_bass_kernel_spmd

S, B, H = 2048, 32, 1024
NCORES = 8
BL = B // NCORES          # batches per core (4)
P = 128                   # SBUF partitions
HT = H // P               # h k-tiles (8)
SC = 512                  # s-chunk per enc DMA
NSC = S // SC             # s chunks per batch row (4)
ST = SC // P              # s-tiles per chunk (4)
HC = H // 512             # h output chunks (2)

BF16 = mybir.dt.bfloat16
F32 = mybir.dt.float32
AFT = mybir.ActivationFunctionType
MUL = mybir.AluOpType.mult
ADD = mybir.AluOpType.add

_nc_cache = None


def build():
    nc = bacc.Bacc()
    enc = nc.declare_dram_parameter("enc", [BL, H, S], BF16, isOutput=False)
    wt = nc.declare_dram_parameter("wt", [2 * H, H], BF16, isOutput=False)
    hid = nc.declare_dram_parameter("hid", [H, BL], BF16, isOutput=False)
    ba = nc.declare_dram_parameter("ba", [1, H], BF16, isOutput=False)
    vv = nc.declare_dram_parameter("v", [1, H], BF16, isOutput=False)
    # out[p, b*16 + sc*4 + st] = softmax row b at s = sc*512 + st*128 + p
    out = nc.declare_dram_parameter("out", [P, BL * NSC * ST], F32, isOutput=True)

    with tile.TileContext(nc) as tc:
        with (
            tc.tile_pool(name="const", bufs=1) as cpool,
            tc.tile_pool(name="encp", bufs=4) as encp,
            tc.tile_pool(name="zqp", bufs=8) as zqp,
            tc.tile_pool(name="alqp", bufs=8) as alqp,
            tc.tile_pool(name="prodp", bufs=4) as prodp,
            tc.tile_pool(name="smallp", bufs=8) as smallp,
            tc.tile_pool(name="zps", bufs=8, space="PSUM") as zps,
        ):
            # --- resident constants ---
            # hid + W1 on the fast ACT hwdge queue, matching the PE stream
            # order (hp matmuls lead): hp trickles from ~8us instead of ~15
            hid_sb = cpool.tile([P, HT, BL], BF16)
            nc.scalar.dma_start(hid_sb[:], hid.rearrange("(t p) b -> p t b", p=P))
            w_sb = cpool.tile([P, 2 * HT, H], BF16)   # W1T | W2T, k on partitions
            for t in range(HT):
                nc.scalar.dma_start(w_sb[:, t, :], wt[t * P:(t + 1) * P, :])
            ba_sb = cpool.tile([1, H], BF16)
            nc.scalar.dma_start(ba_sb[:], ba[:])
            v_row = cpool.tile([1, H], BF16)
            nc.scalar.dma_start(v_row[:], vv[:])
            ones1 = cpool.tile([1, BL], BF16)
            nc.vector.memset(ones1[:], 1.0)
            # sync queue concurrently: enc chunk0 + W2 pairs for the z stream
            et0 = encp.tile([P, HT, SC], BF16, tag="enc")
            for kt in range(HT):   # pairwise: z matmul kt needs both pieces
                nc.sync.dma_start(w_sb[:, HT + kt, :],
                                  wt[(HT + kt) * P:(HT + kt + 1) * P, :])
                nc.sync.dma_start(et0[:, kt, :], enc[0, kt * P:(kt + 1) * P, 0:SC])
            v_bc = cpool.tile([P, H], BF16)
            nc.gpsimd.partition_broadcast(v_bc[:], v_row[:])

            # --- per-batch bias row: hp[b, :] = W1 @ hidden[b] + b_attn ---
            # (emitted after chunk 0's matmuls: W1 arrives after W2)
            hp_bc = []

            def emit_hp():
                hpb16 = cpool.tile([BL, H], BF16)
                for hc in range(HC):
                    hp_ps = zps.tile([BL, 512], F32, tag="z", name=f"hp{hc}")
                    for kt in range(HT):
                        nc.tensor.matmul(
                            hp_ps[:], hid_sb[:, kt, :],
                            w_sb[:, kt, hc * 512:(hc + 1) * 512],
                            start=(kt == 0), stop=False,
                        )
                    # + b_attn as a K=1 rank-1 update (ones ⊗ ba)
                    nc.tensor.matmul(
                        hp_ps[:], ones1[:], ba_sb[:, hc * 512:(hc + 1) * 512],
                        start=False, stop=True,
                    )
                    nc.scalar.copy(hpb16[:, hc * 512:(hc + 1) * 512], hp_ps[:])
                for b in range(BL):
                    # engines can only address partition bases {0,32,64,96};
                    # DMA the row to partition 0 first, then broadcast
                    row = cpool.tile([1, H], BF16, tag=f"hprow{b}")
                    # gpsimd queue: a dependent DMA on the sync queue would
                    # head-of-line-block the enc prefetch stream
                    nc.gpsimd.dma_start(row[:], hpb16[b:b + 1, :])
                    t = cpool.tile([P, H], BF16, tag=f"hpbc{b}")
                    nc.gpsimd.partition_broadcast(t[:], row[:])
                    hp_bc.append(t)

            # --- main loop ---
            scores_sb = cpool.tile([P, BL * NSC * ST], F32)
            ex_sb = cpool.tile([P, BL * NSC * ST], F32)
            osb = cpool.tile([P, BL * NSC * ST], F32)
            first = True
            for b in range(BL):
                for sc in range(NSC):
                    if first:
                        et = et0
                        emit_hp()
                        first = False
                    else:
                        # one 1MB DMA per chunk: the kt-split only matters
                        # for chunk 0's trickle-start; fewer instructions =
                        # less queue/semaphore traffic
                        et = encp.tile([P, HT, SC], BF16, tag="enc")
                        nc.sync.dma_start(
                            et[:],
                            enc[b, :, sc * SC:(sc + 1) * SC].rearrange(
                                "(t p) s -> p t s", p=P),
                        )
                    for st in range(ST):
                        parts = []
                        for hc in range(HC):
                            z_ps = zps.tile([P, 512], F32, tag="z")
                            for kt in range(HT):
                                nc.tensor.matmul(
                                    z_ps[:],
                                    et[:, kt, st * P:(st + 1) * P],
                                    w_sb[:, HT + kt, hc * 512:(hc + 1) * 512],
                                    start=(kt == 0), stop=(kt == HT - 1),
                                )
                            zq = zqp.tile([P, 512], BF16, tag="zq")
                            nc.vector.tensor_add(
                                zq[:], z_ps[:], hp_bc[b][:, hc * 512:(hc + 1) * 512])
                            alq = alqp.tile([P, 512], BF16, tag="alq")
                            nc.scalar.activation(alq[:], zq[:], AFT.Tanh)
                            # fused multiply+reduce on DVE; the elementwise
                            # result is discarded via a step-0 dummy out
                            # (tensor_tensor_reduce crashes this runtime)
                            dummy = prodp.tile([P, 1], BF16, tag="prod")
                            part = smallp.tile([P, 1], F32, tag="part")
                            nc.vector.scalar_tensor_tensor(
                                dummy.broadcast_to(alq.shape), alq[:], 1.0,
                                v_bc[:, hc * 512:(hc + 1) * 512],
                                op0=MUL, op1=MUL, accum_out=part[:])
                            parts.append(part)
                        col = (b * NSC + sc) * ST + st
                        nc.vector.tensor_add(
                            scores_sb[:, col:col + 1], parts[0][:], parts[1][:])

                # --- softmax row b (no max-sub: |scores| <= sum|v| ~ 26) ---
                cs = slice(b * NSC * ST, (b + 1) * NSC * ST)
                psum_row = smallp.tile([P, 1], F32, tag="psrow")
                nc.scalar.activation(
                    ex_sb[:, cs], scores_sb[:, cs], AFT.Exp, accum_out=psum_row[:])
                tot = smallp.tile([P, 1], F32, tag="tot")
                nc.gpsimd.partition_all_reduce(
                    tot[:], psum_row[:], P, ReduceOp.add)
                rec = smallp.tile([P, 1], F32, tag="rec")
                nc.vector.reciprocal(rec[:], tot[:])
                nc.vector.tensor_scalar_mul(osb[:, cs], ex_sb[:, cs], rec[:, 0:1])
                # ACT hwdge queue, for the same head-of-line reason
                nc.scalar.dma_start(out[:, cs], osb[:, cs])
    nc.compile()
    return nc


def _get_nc():
    global _nc_cache
    if _nc_cache is None:
        _nc_cache = build()
    return _nc_cache


def kernel(hidden, encoder_outputs, W_attn, b_attn, v, _trace=False):
    bf16 = ml_dtypes.bfloat16
    hidden = np.asarray(hidden, dtype=np.float32)
    encoder_outputs = np.asarray(encoder_outputs, dtype=np.float32)
    W_attn = np.asarray(W_attn, dtype=np.float32)
    b_attn = np.asarray(b_attn, dtype=np.float32)
    v = np.asarray(v, dtype=np.float32)

    wt = W_attn.T.astype(bf16)                     # [2H, H] contiguous
    hid_t = hidden[0].T.astype(bf16)               # [H, B]
    ba = b_attn.reshape(1, H).astype(bf16)
    vv = v.reshape(1, H).astype(bf16)
    # [B, H, S] b-major, s-contiguous
    enc_t = encoder_outputs.transpose(1, 2, 0).astype(bf16)

    in_maps = []
    for c in range(NCORES):
        bsl = slice(c * BL, (c + 1) * BL)
        in_maps.append({
            "enc": np.ascontiguousarray(enc_t[bsl]),
            "wt": wt,
            "hid": np.ascontiguousarray(hid_t[:, bsl]),
            "ba": ba,
            "v": vv,
        })

    nc = _get_nc()
    res = run_bass_kernel_spmd(
        nc, in_maps, core_ids=list(range(NCORES)), trace=_trace,
    )
    # out[p, b*16 + sc*4 + st] -> [b, s = sc*512 + st*128 + p]
    parts = []
    for c in range(NCORES):
        r = res.results[c]["out"].reshape(P, BL, NSC, ST)
        parts.append(r.transpose(1, 2, 3, 0).reshape(BL, S))
    full = np.concatenate(parts, axis=0)
    out = full[:, None, :].astype(np.float32)      # [B, 1, S]
    if _trace:
        return out, res
    return out



# revision 3
# speedup vs baseline: 1.5563x; 1.5563x over previous
"""Bahdanau-attention scoring kernel for 8 TRN2 NeuronCores (fp8 DoubleRow).

Reference computation (S=2048, B=32, H=1024):
    cat    = concat([broadcast(hidden), enc], axis=2)          # [S,B,2H]
    alphas = tanh(einsum('sbk,hk->sbh', cat, W_attn) + b_attn) # [S,B,H]
    scores = einsum('sbh,h->sb', alphas, v)                    # [S,B]
    out    = softmax(scores.T, axis=1)[:, None, :]             # [B,1,S]

Because hidden broadcasts over S, the concat-matmul splits into
    z[s,b,:] = W2ᵀ enc[s,b,:] + hp[b,:],   hp[b,:] = W1ᵀ hidden[b] + b_attn.

The big matmul (2048·4·1024·1024 MACs per core) runs in fp8 e4m3 with
perf_mode=DoubleRow (2 fp8 MACs per PE cell per cycle).  Host pre-scales
enc×8 and W2×64 to keep values clear of the e4m3 subnormal range; the
1/512 descale rides on the DVE bias-add.  Offline sim with the exact
harness metric: 1.37e-2 mean rel err vs the 2e-2 gate (bf16: 1.5e-3).

Layout: h' (output feature) on partitions, s on the free dim, so that
  - the per-(b,h') bias hp enters as the [P,1] per-partition operand of a
    DVE scalar_tensor_tensor, broadcast along s (zq = z/512 + hp),
  - tanh is a plain ACT op,
  - the v-contraction (over h' = partitions) is a K=128/M=1 PE matmul
    accumulating over the 8 h'-tiles into a [1, 512] PSUM row.
Softmax per batch row runs on one partition ([1, 2048] ACT exp + DVE
reciprocal/scale) — trivial next to the main stream.

Sharding: data-parallel over batch.  Core c handles batches 4c..4c+3.
"""

import sys

for _p in ("/opt/trn_rl_repo", "/root/.axon_site/_ro/trn_rl_repo"):
    if _p not in sys.path:
        sys.path.insert(0, _p)

import numpy as np
import ml_dtypes

import concourse.bass as bass  # noqa: F401  (bass must import before tile)
import concourse.mybir as mybir
import concourse.tile as tile
from concourse import bacc
from concourse.bass_utils import run_bass_kernel_spmd

S, B, H = 2048, 32, 1024
NCORES = 8
BL = B // NCORES          # batches per core (4)
P = 128                   # SBUF partitions
KT2 = H // P              # k-subtiles of 128 (8)
NKT = KT2 // 2            # DoubleRow k-pairs per z tile (4)
HT = H // P               # h'-tiles (8)
SC = 1024                 # s-chunk per enc DMA
NSC = S // SC             # 2
NROW = S // 512           # score rows of 512 per batch (4)

E_SCALE, W_SCALE = 8.0, 64.0
DESCALE = 1.0 / (E_SCALE * W_SCALE)

F8 = mybir.dt.float8e4
BF16 = mybir.dt.bfloat16
F32 = mybir.dt.float32
AFT = mybir.ActivationFunctionType
MUL = mybir.AluOpType.mult
ADD = mybir.AluOpType.add
DR = mybir.MatmulPerfMode.DoubleRow

SKEW = 8                  # how far v-matmuls trail the fp8 stream

_nc_cache = None


def build():
    nc = bacc.Bacc()
    enc = nc.declare_dram_parameter("enc", [BL, H, S], F8, isOutput=False)
    w2 = nc.declare_dram_parameter("w2", [H, H], F8, isOutput=False)
    w1 = nc.declare_dram_parameter("w1", [H, H], BF16, isOutput=False)
    hid = nc.declare_dram_parameter("hid", [H, BL], BF16, isOutput=False)
    ba = nc.declare_dram_parameter("ba", [1, H], BF16, isOutput=False)
    vv = nc.declare_dram_parameter("v", [P, HT], BF16, isOutput=False)
    out = nc.declare_dram_parameter("out", [BL, S], F32, isOutput=True)

    with tile.TileContext(nc) as tc:
        with (
            tc.tile_pool(name="const", bufs=1) as cpool,
            tc.tile_pool(name="encp", bufs=3) as encp,
            tc.tile_pool(name="zqp", bufs=4) as zqp,
            tc.tile_pool(name="alqp", bufs=8) as alqp,
            tc.tile_pool(name="smallp", bufs=2) as smallp,
            tc.tile_pool(name="zps", bufs=3, space="PSUM") as zps,
            tc.tile_pool(name="sps", bufs=2, space="PSUM") as sps,
        ):
            # --- resident constants; w1 split over two queues, w2 on a
            # third, enc chunks stream on the fourth (sync) ---
            hid_sb = cpool.tile([P, KT2, BL], BF16)
            nc.scalar.dma_start(hid_sb[:], hid.rearrange("(t p) b -> p t b", p=P))
            ba_sb = cpool.tile([1, H], BF16)
            nc.scalar.dma_start(ba_sb[:], ba[:])
            v_sb = cpool.tile([P, HT], BF16)
            nc.scalar.dma_start(v_sb[:], vv[:])
            w1_sb = cpool.tile([P, KT2, H], BF16)
            for kt in range(KT2):
                q = nc.scalar if kt < 6 else nc.gpsimd
                q.dma_start(w1_sb[:, kt, :], w1[kt * P:(kt + 1) * P, :])
            w2_sb = cpool.tile([P, KT2, H], F8)
            for kt in range(KT2):
                nc.gpsimd.dma_start(w2_sb[:, kt, :], w2[kt * P:(kt + 1) * P, :])
            ones1 = cpool.tile([1, BL], BF16)
            nc.vector.memset(ones1[:], 1.0)

            # --- hp[b,:] = W1ᵀ hidden[b] + b_attn, h'-major [P, ht, b] ---
            hp_t = cpool.tile([P, HT, BL], F32)
            for ht in range(HT):
                hp_ps = sps.tile([P, BL], F32, tag="srow")
                for kt in range(KT2):
                    nc.tensor.matmul(
                        hp_ps[:], w1_sb[:, kt, ht * P:(ht + 1) * P],
                        hid_sb[:, kt, :], start=(kt == 0), stop=False)
                # + b_attn as a K=1 rank-1 update (ba ⊗ ones)
                nc.tensor.matmul(
                    hp_ps[:], ba_sb[:, ht * P:(ht + 1) * P], ones1[:],
                    start=False, stop=True)
                nc.vector.tensor_copy(hp_t[:, ht, :], hp_ps[:])

            # --- main loop ---
            scores = []
            for b in range(BL):
                srow = cpool.tile([1, S], F32, tag=f"scores{b}",
                                  name=f"scores{b}")
                scores.append(srow)
            # emission FIFO: v-matmuls (and everything downstream) trail
            # the fp8 stream so PE never waits on DVE/ACT results
            pending = []

            def drain(n):
                while len(pending) > n:
                    pending.pop(0)()

            for b in range(BL):
                for sc in range(NSC):
                    et = encp.tile([P, KT2, SC], F8, tag="enc")
                    nc.sync.dma_start(
                        et[:],
                        enc[b, :, sc * SC:(sc + 1) * SC].rearrange(
                            "(t p) s -> p t s", p=P))
                    for half in range(2):
                        row = sc * 2 + half
                        score_ps = sps.tile([1, 512], F32, tag="srow",
                                            name=f"srow{b}_{row}")
                        for htp in range(HT // 2):
                            z_ps = zps.tile([P, 2, 512], F32, tag="z")
                            for ht2 in range(2):
                                ht = htp * 2 + ht2
                                for kt in range(NKT):
                                    nc.tensor.matmul(
                                        z_ps[:, ht2, :],
                                        w2_sb[:, 2 * kt:2 * kt + 2,
                                              ht * P:(ht + 1) * P],
                                        et[:, 2 * kt:2 * kt + 2,
                                           half * 512:(half + 1) * 512],
                                        start=(kt == 0), stop=(kt == NKT - 1),
                                        perf_mode=DR)
                            zq = zqp.tile([P, 2, 512], BF16, tag="zq")
                            nc.vector.scalar_tensor_tensor(
                                zq[:], z_ps[:], DESCALE,
                                hp_t[:, htp * 2:htp * 2 + 2,
                                     b:b + 1].broadcast_to((P, 2, 512)),
                                op0=MUL, op1=ADD)
                            alq = alqp.tile([P, 2, 512], BF16, tag="alq")
                            nc.scalar.activation(alq[:], zq[:], AFT.Tanh)
                            for ht2 in range(2):
                                ht = htp * 2 + ht2

                                def vmm(ht=ht, ht2=ht2, alq=alq,
                                        score_ps=score_ps):
                                    nc.tensor.matmul(
                                        score_ps[:], v_sb[:, ht:ht + 1],
                                        alq[:, ht2, :],
                                        start=(ht == 0), stop=(ht == HT - 1))
                                pending.append(vmm)
                            drain(SKEW)

                        def fin(b=b, row=row, score_ps=score_ps):
                            nc.vector.tensor_copy(
                                scores[b][:, row * 512:(row + 1) * 512],
                                score_ps[:])
                        pending.append(fin)

                # softmax row b (no max-sub: |scores| <= sum|v| ~ 26)
                def softmax(b=b):
                    ex = smallp.tile([1, S], F32, tag="ex")
                    tot = smallp.tile([1, 1], F32, tag="tot", bufs=4)
                    nc.scalar.activation(ex[:], scores[b][:], AFT.Exp,
                                         accum_out=tot[:])
                    rec = smallp.tile([1, 1], F32, tag="rec", bufs=4)
                    nc.vector.reciprocal(rec[:], tot[:])
                    osb = smallp.tile([1, S], F32, tag="osb")
                    nc.vector.tensor_scalar_mul(osb[:], ex[:], rec[:, 0:1])
                    nc.scalar.dma_start(out[b:b + 1, :], osb[:])
                pending.append(softmax)
            drain(0)
    nc.compile()
    return nc


def _get_nc():
    global _nc_cache
    if _nc_cache is None:
        _nc_cache = build()
    return _nc_cache


def kernel(hidden, encoder_outputs, W_attn, b_attn, v, _trace=False):
    f8 = ml_dtypes.float8_e4m3
    bf16 = ml_dtypes.bfloat16
    hidden = np.asarray(hidden, dtype=np.float32)
    encoder_outputs = np.asarray(encoder_outputs, dtype=np.float32)
    W_attn = np.asarray(W_attn, dtype=np.float32)
    b_attn = np.asarray(b_attn, dtype=np.float32)
    v = np.asarray(v, dtype=np.float32)

    w1 = np.ascontiguousarray(W_attn[:, :H].T).astype(bf16)      # [H(k), H(h')]
    w2 = np.ascontiguousarray(W_attn[:, H:].T * W_SCALE).astype(f8)
    hid_t = hidden[0].T.astype(bf16)                             # [H, B]
    ba = b_attn.reshape(1, H).astype(bf16)
    vv = np.ascontiguousarray(v.reshape(HT, P).T).astype(bf16)   # [P, HT]
    # [B, H, S] b-major, s-contiguous, pre-scaled fp8
    enc_t = (encoder_outputs.transpose(1, 2, 0) * E_SCALE).astype(f8)

    in_maps = []
    for c in range(NCORES):
        bsl = slice(c * BL, (c + 1) * BL)
        in_maps.append({
            "enc": np.ascontiguousarray(enc_t[bsl]),
            "w2": w2,
            "w1": w1,
            "hid": np.ascontiguousarray(hid_t[:, bsl]),
            "ba": ba,
            "v": vv,
        })

    nc = _get_nc()
    res = run_bass_kernel_spmd(
        nc, in_maps, core_ids=list(range(NCORES)), trace=_trace,
    )
    parts = [res.results[c]["out"] for c in range(NCORES)]      # [BL, S] each
    full = np.concatenate(parts, axis=0)
    out = full[:, None, :].astype(np.float32)                   # [B, 1, S]
    if _trace:
        return out, res
    return out


# revision 5
# speedup vs baseline: 1.5698x; 1.0086x over previous
"""Bahdanau-attention scoring kernel for 8 TRN2 NeuronCores (fp8 DoubleRow).

Reference computation (S=2048, B=32, H=1024):
    cat    = concat([broadcast(hidden), enc], axis=2)          # [S,B,2H]
    alphas = tanh(einsum('sbk,hk->sbh', cat, W_attn) + b_attn) # [S,B,H]
    scores = einsum('sbh,h->sb', alphas, v)                    # [S,B]
    out    = softmax(scores.T, axis=1)[:, None, :]             # [B,1,S]

Because hidden broadcasts over S, the concat-matmul splits into
    z[s,b,:] = W2ᵀ enc[s,b,:] + hp[b,:],   hp[b,:] = W1ᵀ hidden[b] + b_attn.

The big matmul (2048·4·1024·1024 MACs per core) runs in fp8 e4m3 with
perf_mode=DoubleRow (2 fp8 MACs per PE cell per cycle).  Host pre-scales
enc×8 and W2×64 to keep values clear of the e4m3 subnormal range; the
1/512 descale rides on the DVE bias-add.  Offline sim with the exact
harness metric: 1.37e-2 mean rel err vs the 2e-2 gate (bf16: 1.5e-3).

Layout: h' (output feature) on partitions, s on the free dim, so that
  - the per-(b,h') bias hp enters as the [P,1] per-partition operand of a
    DVE scalar_tensor_tensor, broadcast along s (zq = z/512 + hp),
  - tanh is a plain ACT op,
  - the v-contraction (over h' = partitions) is a K=128/M=1 PE matmul.
    The 8 h'-tiles map to PE column groups 0/32/64/96 (explicit
    tile_position col-tiling) so 4 of these run concurrently in the
    array; the 4 partial rows are summed on the (otherwise idle) DVE.

Schedule notes:
  - v-matmuls are emitted in quads through a pending-FIFO that trails
    the fp8 stream by ~1 s-half, so the PE never waits on DVE/ACT.
  - hp's 8 matmul groups interleave with the first s-chunk's fp8 groups
    and w1 arrives in h'-major 256KB chunks, so the prologue is DMA-rate
    limited instead of serialized behind the full 2MB w1 load.
  - softmax exp runs per 512-row as scores complete; only the tiny
    total/reciprocal/scale work trails the last matmul.

Sharding: data-parallel over batch.  Core c handles batches 4c..4c+3.
"""

import sys

for _p in ("/opt/trn_rl_repo", "/root/.axon_site/_ro/trn_rl_repo"):
    if _p not in sys.path:
        sys.path.insert(0, _p)

import numpy as np
import ml_dtypes

import concourse.bass as bass  # noqa: F401  (bass must import before tile)
import concourse.mybir as mybir
import concourse.tile as tile
from concourse import bacc
from concourse.bass_utils import run_bass_kernel_spmd

S, B, H = 2048, 32, 1024
NCORES = 8
BL = B // NCORES          # batches per core (4)
P = 128                   # SBUF partitions
KT2 = H // P              # k-subtiles of 128 (8)
NKT = KT2 // 2            # DoubleRow k-pairs per z tile (4)
HT = H // P               # h'-tiles (8)
SC = 1024                 # s-chunk per enc DMA
NSC = S // SC             # 2
NROW = S // 512           # score rows of 512 per batch (4)

E_SCALE, W_SCALE = 8.0, 64.0
DESCALE = 1.0 / (E_SCALE * W_SCALE)

F8 = mybir.dt.float8e4
BF16 = mybir.dt.bfloat16
F32 = mybir.dt.float32
AFT = mybir.ActivationFunctionType
MUL = mybir.AluOpType.mult
ADD = mybir.AluOpType.add
DR = mybir.MatmulPerfMode.DoubleRow

SKEW = 5                  # pending-FIFO depth (quads/fins trailing the fp8 stream)

_nc_cache = None


def build():
    nc = bacc.Bacc()
    enc = nc.declare_dram_parameter("enc", [BL, H, S], F8, isOutput=False)
    w2 = nc.declare_dram_parameter("w2", [H, H], F8, isOutput=False)
    w1h = nc.declare_dram_parameter("w1h", [HT, P, H], BF16, isOutput=False)
    hid = nc.declare_dram_parameter("hid", [H, BL], BF16, isOutput=False)
    ba = nc.declare_dram_parameter("ba", [1, H], BF16, isOutput=False)
    vv = nc.declare_dram_parameter("v", [P, HT], BF16, isOutput=False)
    out = nc.declare_dram_parameter("out", [BL, S], F32, isOutput=True)

    with tile.TileContext(nc) as tc:
        with (
            tc.tile_pool(name="const", bufs=1) as cpool,
            tc.tile_pool(name="encp", bufs=3) as encp,
            tc.tile_pool(name="zqp", bufs=4) as zqp,
            tc.tile_pool(name="alqp", bufs=12) as alqp,
            tc.tile_pool(name="smallp", bufs=2) as smallp,
            tc.tile_pool(name="zps", bufs=3, space="PSUM") as zps,
            tc.tile_pool(name="sps", bufs=2, space="PSUM") as sps,
        ):
            # --- resident constants across three DMA queues ---
            hid_sb = cpool.tile([P, KT2, BL], BF16)
            nc.scalar.dma_start(hid_sb[:], hid.rearrange("(t p) b -> p t b", p=P))
            ba_sb = cpool.tile([1, H], BF16)
            nc.scalar.dma_start(ba_sb[:], ba[:])
            v_sb = cpool.tile([P, HT], BF16)
            nc.scalar.dma_start(v_sb[:], vv[:])
            w1_sb = cpool.tile([P, HT, H], BF16)     # [p, ht, kt*128+m]
            for ht in range(HT):
                q = nc.scalar if ht < 6 else nc.gpsimd
                q.dma_start(w1_sb[:, ht, :], w1h[ht])
            w2_sb = cpool.tile([P, KT2, H], F8)
            for kt in range(KT2):
                nc.gpsimd.dma_start(w2_sb[:, kt, :], w2[kt * P:(kt + 1) * P, :])
            ones1 = cpool.tile([1, BL], BF16)
            nc.vector.memset(ones1[:], 1.0)
            hp_t = cpool.tile([P, HT, BL], F32)      # hp, h'-major

            def emit_hp(ht):
                # hp[b,:] = W1ᵀ hidden[b] + b_attn for one h'-tile
                hp_ps = sps.tile([P, BL], F32, tag="srow", name=f"hp{ht}")
                for kt in range(KT2):
                    nc.tensor.matmul(
                        hp_ps[:], w1_sb[:, ht, kt * P:(kt + 1) * P],
                        hid_sb[:, kt, :], start=(kt == 0), stop=False)
                # + b_attn as a K=1 rank-1 update (ba ⊗ ones)
                nc.tensor.matmul(
                    hp_ps[:], ba_sb[:, ht * P:(ht + 1) * P], ones1[:],
                    start=False, stop=True)
                nc.vector.tensor_copy(hp_t[:, ht, :], hp_ps[:])

            # --- main loop ---
            pending = []

            def drain(n):
                while len(pending) > n:
                    pending.pop(0)()

            for b in range(BL):
                ex = smallp.tile([1, S], F32, tag="ex", name=f"ex{b}")
                tots = []
                for sc in range(NSC):
                    et = encp.tile([P, KT2, SC], F8, tag="enc")
                    for kt in range(NKT):   # kt-pair granularity: MMs can
                        nc.sync.dma_start(  # start before the full chunk lands
                            et[:, 2 * kt:2 * kt + 2, :],
                            enc[b, 2 * kt * P:(2 * kt + 2) * P,
                                sc * SC:(sc + 1) * SC].rearrange(
                                    "(t p) s -> p t s", p=P))
                    for half in range(2):
                        row = sc * 2 + half
                        first = b == 0 and sc == 0 and half == 0
                        score_ps = sps.tile([P, 512], F32, tag="srow",
                                            name=f"srow{b}_{row}")
                        quad = []
                        for htp in range(HT // 2):
                            if first:   # hp rides the first half's fp8 stream
                                emit_hp(htp * 2)
                                emit_hp(htp * 2 + 1)
                            z_ps = zps.tile([P, 2, 512], F32, tag="z")
                            for ht2 in range(2):
                                ht = htp * 2 + ht2
                                for kt in range(NKT):
                                    nc.tensor.matmul(
                                        z_ps[:, ht2, :],
                                        w2_sb[:, 2 * kt:2 * kt + 2,
                                              ht * P:(ht + 1) * P],
                                        et[:, 2 * kt:2 * kt + 2,
                                           half * 512:(half + 1) * 512],
                                        start=(kt == 0), stop=(kt == NKT - 1),
                                        perf_mode=DR)
                            zq = zqp.tile([P, 2, 512], BF16, tag="zq")
                            nc.vector.scalar_tensor_tensor(
                                zq[:], z_ps[:], DESCALE,
                                hp_t[:, htp * 2:htp * 2 + 2,
                                     b:b + 1].broadcast_to((P, 2, 512)),
                                op0=MUL, op1=ADD)
                            alq = alqp.tile([P, 2, 512], BF16, tag="alq")
                            nc.scalar.activation(alq[:], zq[:], AFT.Tanh)
                            quad.append((htp, alq))
                            if htp % 2 == 1:
                                # v-contraction: 4 concurrent M=1 matmuls on
                                # PE column groups 0/32/64/96 — slot j owns
                                # s-quarter j and accumulates all 8 h'-tiles,
                                # so no cross-slot sum is needed
                                def vmms(quad=tuple(quad), score_ps=score_ps):
                                    for hq, a in quad:
                                        for ht2 in range(2):
                                            ht = hq * 2 + ht2
                                            for j in range(4):
                                                nc.tensor.matmul(
                                                    score_ps[32 * j:32 * j + 1,
                                                             0:P],
                                                    v_sb[:, ht:ht + 1],
                                                    a[:, ht2, j * P:(j + 1) * P],
                                                    start=(ht == 0),
                                                    stop=(ht == HT - 1),
                                                    tile_position=(0, 32 * j))
                                pending.append(vmms)
                                quad = []
                        # exp each s-quarter straight out of its column-group
                        # slot; per-row total via a small add tree
                        def fin(b=b, row=row, score_ps=score_ps, ex=ex,
                                tots=tots):
                            ts = []
                            for j in range(4):
                                tj = smallp.tile([1, 1], F32, tag="tot",
                                                 bufs=10, name=f"t{b}_{row}_{j}")
                                nc.scalar.activation(
                                    ex[:, row * 512 + j * P:
                                       row * 512 + (j + 1) * P],
                                    score_ps[32 * j:32 * j + 1, 0:P],
                                    AFT.Exp, accum_out=tj[:])
                                ts.append(tj)
                            t01 = smallp.tile([1, 1], F32, tag="t01", bufs=3)
                            nc.vector.tensor_add(t01[:], ts[0][:], ts[1][:])
                            t23 = smallp.tile([1, 1], F32, tag="t23", bufs=3)
                            nc.vector.tensor_add(t23[:], ts[2][:], ts[3][:])
                            tot = smallp.tile([1, 1], F32, tag="trow", bufs=6,
                                              name=f"tot{b}_{row}")
                            nc.vector.tensor_add(tot[:], t01[:], t23[:])
                            tots.append(tot)
                        pending.append(fin)

                # softmax scale for row b (no max-sub: |scores| <= sum|v| ~ 26)
                def softmax(b=b, ex=ex, tots=tots):
                    t01 = smallp.tile([1, 1], F32, tag="st01", bufs=2)
                    nc.vector.tensor_add(t01[:], tots[0][:], tots[1][:])
                    t23 = smallp.tile([1, 1], F32, tag="st23", bufs=2)
                    nc.vector.tensor_add(t23[:], tots[2][:], tots[3][:])
                    tot = smallp.tile([1, 1], F32, tag="stot", bufs=2)
                    nc.vector.tensor_add(tot[:], t01[:], t23[:])
                    rec = smallp.tile([1, 1], F32, tag="rec", bufs=2)
                    nc.vector.reciprocal(rec[:], tot[:])
                    osb = smallp.tile([1, S], F32, tag="osb")
                    nc.vector.tensor_scalar_mul(osb[:], ex[:], rec[:, 0:1])
                    nc.scalar.dma_start(out[b:b + 1, :], osb[:])
                pending.append(softmax)
            drain(0)
    nc.compile()
    return nc


def _get_nc():
    global _nc_cache
    if _nc_cache is None:
        _nc_cache = build()
    return _nc_cache


def kernel(hidden, encoder_outputs, W_attn, b_attn, v, _trace=False):
    f8 = ml_dtypes.float8_e4m3
    bf16 = ml_dtypes.bfloat16
    hidden = np.asarray(hidden, dtype=np.float32)
    encoder_outputs = np.asarray(encoder_outputs, dtype=np.float32)
    W_attn = np.asarray(W_attn, dtype=np.float32)
    b_attn = np.asarray(b_attn, dtype=np.float32)
    v = np.asarray(v, dtype=np.float32)

    w1t = W_attn[:, :H].T                                        # [k, h']
    # h'-major 256KB chunks: w1h[ht, p, kt*128+m] = W1[kt*128+p, ht*128+m]
    w1h = np.ascontiguousarray(
        w1t.reshape(KT2, P, HT, P).transpose(2, 1, 0, 3).reshape(HT, P, H)
    ).astype(bf16)
    w2 = np.ascontiguousarray(W_attn[:, H:].T * W_SCALE).astype(f8)
    hid_t = hidden[0].T.astype(bf16)                             # [H, B]
    ba = b_attn.reshape(1, H).astype(bf16)
    vv = np.ascontiguousarray(v.reshape(HT, P).T).astype(bf16)   # [P, HT]
    # [B, H, S] b-major, s-contiguous, pre-scaled fp8
    enc_t = (encoder_outputs.transpose(1, 2, 0) * E_SCALE).astype(f8)

    in_maps = []
    for c in range(NCORES):
        bsl = slice(c * BL, (c + 1) * BL)
        in_maps.append({
            "enc": np.ascontiguousarray(enc_t[bsl]),
            "w2": w2,
            "w1h": w1h,
            "hid": np.ascontiguousarray(hid_t[:, bsl]),
            "ba": ba,
            "v": vv,
        })

    nc = _get_nc()
    res = run_bass_kernel_spmd(
        nc, in_maps, core_ids=list(range(NCORES)), trace=_trace,
    )
    parts = [res.results[c]["out"] for c in range(NCORES)]      # [BL, S] each
    full = np.concatenate(parts, axis=0)
    out = full[:, None, :].astype(np.float32)                   # [B, 1, S]
    if _trace:
        return out, res
    return out
